# revision 26
# baseline (speedup 1.0000x reference)
"""AdaptiveSampler Trainium2 kernel (8 NeuronCores, pure data parallel).

Reference computation per batch row b:
    Q  = target_embed @ Wq.T + bq
    K  = candidate_embeds @ Wk.T + bk
    scores[b, n] = (Q[b] . K[b, n]) / sqrt(d)
    probs = 0.9 * softmax(scores) + 0.1 / N_CAND
    keys  = log(probs) + gumbel(u)
    out   = top-32 indices of keys (descending)

Rewrite: scores[b,n] = cand[b,n,:] . qk[b,:] with qk = (Q @ Wk) / sqrt(d)
(the Q.bk term is constant per row and cancels in softmax).  qk is tiny
([B,128]) and precomputed on the host; the device kernel streams the
134 MB/core of candidate embeddings exactly once (memory bound).

Each [128 rows, 128 cands, 128 d] chunk is processed by three engines
cooperatively:
  - PE accumulates d < pe_d via diagonal-weight matmuls into PSUM
    (diag weights built per block from qk x identity on DVE),
  - GPSIMD broadcast-multiplies cand * qk for most of d >= pe_d,
    DVE for the rest, into tmp slabs,
  - DVE segment-reduces the slabs into s_t and adds the PSUM partial.

Scheduling (v3): the Gumbel transform of u runs on ACT right at block
start (it only needs u), and the next block's diag weights are built
on DVE before this block's epilogue is issued, so the PE/DMA pipeline
restarts immediately while DVE chews through softmax + top-k.

Per-block epilogue: fused exp/sum softmax (ACT), mixed probs, log,
Gumbel keys, then top-32 via 4 rounds of max8/max_index/match_replace
(DVE).

Sharding: batch dim 4096 split across 8 cores (512 rows each); no
cross-core communication.
"""

import sys

for _p in ("/opt/trn_rl_repo",):
    if _p not in sys.path:
        sys.path.append(_p)

from contextlib import ExitStack

import numpy as np

import concourse.bacc as bacc
import concourse.mybir as mybir
import concourse.tile as tile
from concourse import masks
from concourse.bass_utils import run_bass_kernel_spmd

F32 = mybir.dt.float32
U32 = mybir.dt.uint32
AF = mybir.ActivationFunctionType
OP = mybir.AluOpType
AX = mybir.AxisListType

B_FULL = 4096
N_CORES = 8
B_SHARD = B_FULL // N_CORES  # 512
D = 128
N_CAND = 512
K_OUT = 32
GAMMA = 0.1
MIX = GAMMA / N_CAND
INVSCALE = float(D) ** -0.5  # folded into qk on the host
NEG_BIG = -1e30


def build_nc(
    b_shard=B_SHARD, pe_d=46, act_d=0, slab=32, dve_slabs=1, ps_bufs=3,
    tmp_bufs=3, act_bufs=2, dq_bufs=1, cand_bufs=2, nch=128, epi_delay=2,
):
    """Build the single-core Bass program (SPMD across 8 cores).

    Inputs: qk [b_shard, 128] fp32 (host-precomputed (Q @ Wk)/sqrt(d)),
    candidate_embeds, u.  Output: top-32 indices as uint32.

    d-range split: [0, pe_d) on PE (diag matmuls); [pe_d, pe_d+act_d)
    on ACT (per-d Copy with per-partition scale = qk[:, d]);
    [pe_d+act_d, 128) broadcast-mult on GPSIMD/DVE slabs.  DVE reduces
    everything non-PE and combines partials.
    """
    assert b_shard % 128 == 0
    nblk = b_shard // 128
    nchunks = N_CAND // nch
    gd0 = pe_d + act_d  # start of the GPSIMD/DVE d-range
    rem_d = D - gd0

    nc = bacc.Bacc("TRN2", target_bir_lowering=False, debug=False)

    t_qk = nc.dram_tensor("qk", [b_shard, D], F32, kind="ExternalInput")
    t_cand = nc.dram_tensor(
        "candidate_embeds", [b_shard, N_CAND, D], F32, kind="ExternalInput"
    )
    t_u = nc.dram_tensor("u", [b_shard, N_CAND], F32, kind="ExternalInput")
    t_out = nc.dram_tensor("out", [b_shard, K_OUT], U32, kind="ExternalOutput")

    cand_ap = t_cand.ap()
    u_ap = t_u.ap()
    out_ap = t_out.ap()

    with tile.TileContext(nc) as tc, ExitStack() as ctx:
        const_pool = ctx.enter_context(tc.tile_pool(name="const", bufs=1))
        psum_pool = ctx.enter_context(tc.tile_pool(name="psum", bufs=1, space="PSUM"))
        cand_pool = ctx.enter_context(tc.tile_pool(name="cand", bufs=cand_bufs))
        work_pool = ctx.enter_context(tc.tile_pool(name="work", bufs=2))

        ident = const_pool.tile([128, 128], F32)
        masks.make_identity(nc, ident[:])

        eps_c = const_pool.tile([128, 1], F32)
        nc.gpsimd.memset(eps_c[:], 1e-20)

        # qk with rows in partitions: qk_all[p, blk*128 + d] = qk[blk*128+p, d]
        qk_all = const_pool.tile([128, b_shard], F32)
        for blk in range(nblk):
            nc.scalar.dma_start(
                qk_all[:, blk * 128 : (blk + 1) * 128],
                t_qk.ap()[blk * 128 : (blk + 1) * 128, :],
            )

        def build_dq(bb):
            """Diag weights for block bb: dq[p, j, :] = qk[p, j] * I[p, :]."""
            qk_blk = qk_all[:, bb * 128 : bb * 128 + 128]
            dq_t = work_pool.tile([128, pe_d, 128], F32, tag="dq_t", bufs=dq_bufs)
            nc.vector.tensor_tensor(
                dq_t[:],
                qk_blk[:, :pe_d][:, :, None].to_broadcast([128, pe_d, 128]),
                ident[:][:, None, :].to_broadcast([128, pe_d, 128]),
                op=OP.mult,
            )
            return dq_t

        dq_cur = build_dq(0)

        def make_epilogue(bb, s_t, l2_t, split_out=False):
            """Emit softmax + top-k for block bb as two sub-bursts, dropped
            into the next block's chunk stream so the DVE queue never backs
            up far enough to block the streaming pipeline's buffer reuse."""
            state = {}

            def part1():
                # s is already scaled by 1/sqrt(d) (folded into qk on host)
                m_t = work_pool.tile([128, 1], F32, tag="m_t")
                nc.vector.tensor_reduce(m_t[:], s_t[:], axis=AX.X, op=OP.max)
                mb_t = work_pool.tile([128, 1], F32, tag="mb_t")
                nc.vector.tensor_scalar_mul(mb_t[:], m_t[:], -1.0)

                e_t = psum_pool.tile([128, N_CAND], F32, tag="e_t")
                sum_t = work_pool.tile([128, 1], F32, tag="sum_t")
                nc.scalar.activation(
                    e_t[:], s_t[:], AF.Exp, bias=mb_t[:], scale=1.0,
                    accum_out=sum_t[:],
                )
                r_t = work_pool.tile([128, 1], F32, tag="r_t")
                nc.vector.reciprocal(r_t[:], sum_t[:])
                r9_t = work_pool.tile([128, 1], F32, tag="r9_t")
                nc.vector.tensor_scalar_mul(r9_t[:], r_t[:], 1.0 - GAMMA)
                # p = e * (0.9/sum) + GAMMA/N_CAND  (in place in PSUM)
                nc.vector.tensor_scalar(
                    e_t[:], e_t[:], r9_t[:], MIX, op0=OP.mult, op1=OP.add
                )
                lp_t = psum_pool.tile([128, N_CAND], F32, tag="lp_t")
                nc.scalar.activation(lp_t[:], e_t[:], AF.Ln)

                # keys = log(p) + g = lp - l2  (write over s_t, now dead)
                nc.vector.tensor_sub(s_t[:], lp_t[:], l2_t[:])
                keys_t = s_t

                # top-32 round 0
                idx_t = work_pool.tile([128, K_OUT], U32, tag="idx_t")
                m8_t = work_pool.tile([128, 8], F32, tag="m8_t")
                nc.vector.max(out=m8_t[:], in_=keys_t[:])
                nc.vector.max_index(
                    out=idx_t[:, 0:8], in_max=m8_t[:], in_values=keys_t[:]
                )
                nc.vector.match_replace(
                    out=keys_t[:], in_to_replace=m8_t[:], in_values=keys_t[:],
                    imm_value=NEG_BIG,
                )
                if split_out:
                    nc.scalar.dma_start(
                        out_ap[bb * 128 : bb * 128 + 128, 0:8], idx_t[:, 0:8]
                    )
                state.update(keys_t=keys_t, idx_t=idx_t, m8_t=m8_t)

            def part2():
                keys_t, idx_t, m8_t = (
                    state["keys_t"], state["idx_t"], state["m8_t"]
                )
                for r in range(1, K_OUT // 8):
                    nc.vector.max(out=m8_t[:], in_=keys_t[:])
                    nc.vector.max_index(
                        out=idx_t[:, r * 8 : (r + 1) * 8],
                        in_max=m8_t[:],
                        in_values=keys_t[:],
                    )
                    if r < K_OUT // 8 - 1:
                        nc.vector.match_replace(
                            out=keys_t[:],
                            in_to_replace=m8_t[:],
                            in_values=keys_t[:],
                            imm_value=NEG_BIG,
                        )
                    if split_out:
                        nc.scalar.dma_start(
                            out_ap[bb * 128 : bb * 128 + 128, r * 8 : (r + 1) * 8],
                            idx_t[:, r * 8 : (r + 1) * 8],
                        )

                if not split_out:
                    nc.scalar.dma_start(
                        out_ap[bb * 128 : bb * 128 + 128, :], idx_t[:]
                    )

            return [part1, part2]

        pending_epi = []

        # ---------------- main loop over 128-row blocks ------------------------
        for bb in range(nblk):
            r0 = bb * 128
            u_t = work_pool.tile([128, N_CAND], F32, tag="u_t")
            nc.scalar.dma_start(u_t[:], u_ap[r0 : r0 + 128, :])

            # gumbel early (only needs u; ACT is idle during streaming):
            # l2 = log(-log(u + 1e-20) + 1e-20); keys later use g = -l2
            l1_t = psum_pool.tile([128, N_CAND], F32, tag="l1_t")
            nc.scalar.activation(l1_t[:], u_t[:], AF.Ln, bias=eps_c[:], scale=1.0)
            l2_t = u_t  # u is dead; keep l2 in SBUF (DVE reads one PSUM input max)
            nc.scalar.activation(l2_t[:], l1_t[:], AF.Ln, bias=eps_c[:], scale=-1.0)
            if bb == nblk - 1:
                # touch Exp now so the epilogue's exp finds its ACT table
                # loaded (the load otherwise lands in the post-DMA tail)
                warm_t = work_pool.tile([128, 1], F32, tag="warm_t")
                nc.scalar.activation(warm_t[:], eps_c[:], AF.Exp)

            s_t = work_pool.tile([128, N_CAND], F32, tag="s_t")
            qk_blk = qk_all[:, r0 : r0 + 128]
            dq_t = dq_cur

            # last block streams a finer tail so the final chunk is small and
            # PE-free (elementwise engines chase the DMA right to the end)
            if bb == nblk - 1 and nch == 128 and slab == 32:
                chunk_list = [(0, 128, True), (128, 128, True), (256, 128, True),
                              (384, 96, True), (480, 32, False)]
            else:
                chunk_list = [(ch * nch, nch, True) for ch in range(nchunks)]

            last_blk = bb == nblk - 1
            for ch, (n0, nch_c, use_pe) in enumerate(chunk_list):
                cand_t = cand_pool.tile([128, nch, D], F32, tag="cand_t")
                nc.sync.dma_start(
                    cand_t[:, :nch_c, :], cand_ap[r0 : r0 + 128, n0 : n0 + nch_c, :]
                )
                seg = s_t[:, n0 : n0 + nch_c]

                if use_pe:
                    # PE: partial scores over d < pe_d, accumulated in PSUM
                    ps_t = psum_pool.tile([128, nch], F32, tag="ps_mm", bufs=ps_bufs)
                    for dd in range(pe_d):
                        nc.tensor.matmul(
                            ps_t[:, :nch_c],
                            dq_t[:, dd, :],
                            cand_t[:, :nch_c, dd],
                            start=(dd == 0),
                            stop=(dd == pe_d - 1),
                        )

                    # GPSIMD/DVE multiply + DVE segmented reduce for d >= gd0
                    for hi in range(nch_c // slab):
                        h = hi * slab
                        tmp_t = work_pool.tile(
                            [128, slab, rem_d], F32, tag="tmp_t", bufs=tmp_bufs
                        )
                        mul_eng = (
                            nc.vector
                            if hi < dve_slabs and not last_blk
                            else nc.gpsimd
                        )
                        mul_eng.tensor_tensor(
                            tmp_t[:],
                            cand_t[:, h : h + slab, gd0:],
                            qk_blk[:, None, gd0:].to_broadcast([128, slab, rem_d]),
                            op=OP.mult,
                        )
                        nc.vector.tensor_reduce(
                            seg[:, h : h + slab], tmp_t[:], axis=AX.X, op=OP.add
                        )

                    # combine: seg += PE partial
                    nc.vector.tensor_tensor(seg, seg, ps_t[:, :nch_c], op=OP.add)
                else:
                    # PE-free tail chunk (32 cands): GPSIMD takes d < rem_d
                    # with a contiguous full-width tmp write, DVE the trailing
                    # pe_d-sized range; both reduced on DVE
                    d1 = rem_d
                    tmp_a = work_pool.tile(
                        [128, slab, rem_d], F32, tag="tmp_t", bufs=tmp_bufs
                    )
                    nc.gpsimd.tensor_tensor(
                        tmp_a[:],
                        cand_t[:, :nch_c, :d1],
                        qk_blk[:, None, :d1].to_broadcast([128, nch_c, d1]),
                        op=OP.mult,
                    )
                    tmp_b = work_pool.tile(
                        [128, slab, rem_d], F32, tag="tmp_t", bufs=tmp_bufs
                    )
                    nc.vector.tensor_tensor(
                        tmp_b[:, :, : D - d1],
                        cand_t[:, :nch_c, d1:],
                        qk_blk[:, None, d1:].to_broadcast([128, nch_c, D - d1]),
                        op=OP.mult,
                    )
                    nc.vector.tensor_reduce(
                        seg, tmp_a[:], axis=AX.X, op=OP.add
                    )
                    seg_b = work_pool.tile([128, slab], F32, tag="seg_b")
                    nc.vector.tensor_reduce(
                        seg_b[:, :nch_c], tmp_b[:, :, : D - d1], axis=AX.X,
                        op=OP.add,
                    )
                    nc.vector.tensor_tensor(seg, seg, seg_b[:, :nch_c], op=OP.add)

                # previous block's delayed epilogue parts drop in here, after
                # this block's streaming has refilled the DVE queue
                if ch >= 1 and pending_epi:
                    pending_epi.pop(0)()

            # prepare next block's weights right after this block's chunks so
            # the PE pipeline restarts immediately
            if bb + 1 < nblk:
                dq_cur = build_dq(bb + 1)

            while pending_epi:
                pending_epi.pop(0)()
            pending_epi = make_epilogue(bb, s_t, l2_t, split_out=(bb == nblk - 1))

        while pending_epi:
            pending_epi.pop(0)()

    nc.compile()
    return nc


_CACHE = {}


def _get_nc():
    if "nc" not in _CACHE:
        _CACHE["nc"] = build_nc()
    return _CACHE["nc"]


def make_in_maps(target_embed, candidate_embeds, Wq, bq, Wk, bk, u):
    target_embed = np.ascontiguousarray(np.asarray(target_embed, dtype=np.float32))
    candidate_embeds = np.ascontiguousarray(
        np.asarray(candidate_embeds, dtype=np.float32)
    )
    Wq = np.asarray(Wq, dtype=np.float32)
    bq = np.asarray(bq, dtype=np.float32)
    Wk = np.asarray(Wk, dtype=np.float32)
    u = np.ascontiguousarray(np.asarray(u, dtype=np.float32))

    # Host-side projection (tiny): qk = ((target @ Wq.T + bq) @ Wk) / sqrt(d)
    q = target_embed @ Wq.T + bq
    qk = np.ascontiguousarray(((q @ Wk) * INVSCALE).astype(np.float32))

    in_maps = []
    for c in range(N_CORES):
        lo, hi = c * B_SHARD, (c + 1) * B_SHARD
        in_maps.append(
            {
                "qk": qk[lo:hi],
                "candidate_embeds": candidate_embeds[lo:hi],
                "u": u[lo:hi],
            }
        )
    return in_maps


def kernel(
    target_embed, candidate_embeds, Wq, bq, Wk, bk, u
):  # full inputs -> full output
    nc = _get_nc()
    in_maps = make_in_maps(target_embed, candidate_embeds, Wq, bq, Wk, bk, u)
    res = run_bass_kernel_spmd(nc, in_maps, core_ids=list(range(N_CORES)))
    outs = [r["out"].astype(np.int32) for r in res.results]
    return np.concatenate(outs, axis=0)


# revision 27
# speedup vs baseline: 1.0468x; 1.0468x over previous
"""AdaptiveSampler Trainium2 kernel (8 NeuronCores, pure data parallel).

Reference computation per batch row b:
    Q  = target_embed @ Wq.T + bq
    K  = candidate_embeds @ Wk.T + bk
    scores[b, n] = (Q[b] . K[b, n]) / sqrt(d)
    probs = 0.9 * softmax(scores) + 0.1 / N_CAND
    keys  = log(probs) + gumbel(u)
    out   = top-32 indices of keys (descending)

Rewrite: scores[b,n] = cand[b,n,:] . qk[b,:] with qk = (Q @ Wk) / sqrt(d)
(the Q.bk term is constant per row and cancels in softmax).  qk is tiny
([B,128]) and precomputed on the host; the device kernel streams the
134 MB/core of candidate embeddings exactly once (memory bound).

Each [128 rows, 128 cands, 128 d] chunk is processed by three engines
cooperatively:
  - PE accumulates d < pe_d via diagonal-weight matmuls into PSUM
    (diag weights built per block from qk x identity on DVE),
  - GPSIMD broadcast-multiplies cand * qk for most of d >= pe_d,
    DVE for the rest, into tmp slabs,
  - DVE segment-reduces the slabs into s_t and adds the PSUM partial.

Scheduling (v3): the Gumbel transform of u runs on ACT right at block
start (it only needs u), and the next block's diag weights are built
on DVE before this block's epilogue is issued, so the PE/DMA pipeline
restarts immediately while DVE chews through softmax + top-k.

Per-block epilogue: fused exp/sum softmax (ACT), mixed probs, log,
Gumbel keys, then top-32 via 4 rounds of max8/max_index/match_replace
(DVE).

Sharding: batch dim 4096 split across 8 cores (512 rows each); no
cross-core communication.
"""

import sys

for _p in ("/opt/trn_rl_repo",):
    if _p not in sys.path:
        sys.path.append(_p)

from contextlib import ExitStack

import numpy as np

import concourse.bacc as bacc
import concourse.mybir as mybir
import concourse.tile as tile
from concourse import masks
from concourse.bass_utils import run_bass_kernel_spmd

F32 = mybir.dt.float32
U32 = mybir.dt.uint32
AF = mybir.ActivationFunctionType
OP = mybir.AluOpType
AX = mybir.AxisListType

B_FULL = 4096
N_CORES = 8
B_SHARD = B_FULL // N_CORES  # 512
D = 128
N_CAND = 512
K_OUT = 32
GAMMA = 0.1
MIX = GAMMA / N_CAND
INVSCALE = float(D) ** -0.5  # folded into qk on the host
NEG_BIG = -1e30


def build_nc(
    b_shard=B_SHARD, pe_d=46, act_d=0, slab=32, dve_slabs=1, ps_bufs=3,
    tmp_bufs=3, act_bufs=2, dq_bufs=1, cand_bufs=2, nch=128, epi_delay=2,
):
    """Build the single-core Bass program (SPMD across 8 cores).

    Inputs: qk [b_shard, 128] fp32 (host-precomputed (Q @ Wk)/sqrt(d)),
    candidate_embeds, u.  Output: top-32 indices as uint32.

    d-range split: [0, pe_d) on PE (diag matmuls); [pe_d, pe_d+act_d)
    on ACT (per-d Copy with per-partition scale = qk[:, d]);
    [pe_d+act_d, 128) broadcast-mult on GPSIMD/DVE slabs.  DVE reduces
    everything non-PE and combines partials.
    """
    assert b_shard % 128 == 0
    nblk = b_shard // 128
    nchunks = N_CAND // nch
    gd0 = pe_d + act_d  # start of the GPSIMD/DVE d-range
    rem_d = D - gd0

    nc = bacc.Bacc("TRN2", target_bir_lowering=False, debug=False)

    t_qk = nc.dram_tensor("qk", [b_shard, D], F32, kind="ExternalInput")
    t_cand = nc.dram_tensor(
        "candidate_embeds", [b_shard, N_CAND, D], F32, kind="ExternalInput"
    )
    t_u = nc.dram_tensor("u", [b_shard, N_CAND], F32, kind="ExternalInput")
    t_out = nc.dram_tensor("out", [b_shard, K_OUT], U32, kind="ExternalOutput")

    cand_ap = t_cand.ap()
    u_ap = t_u.ap()
    out_ap = t_out.ap()

    with tile.TileContext(nc) as tc, ExitStack() as ctx:
        const_pool = ctx.enter_context(tc.tile_pool(name="const", bufs=1))
        psum_pool = ctx.enter_context(tc.tile_pool(name="psum", bufs=1, space="PSUM"))
        cand_pool = ctx.enter_context(tc.tile_pool(name="cand", bufs=cand_bufs))
        work_pool = ctx.enter_context(tc.tile_pool(name="work", bufs=2))

        ident = const_pool.tile([128, 128], F32)
        masks.make_identity(nc, ident[:])

        eps_c = const_pool.tile([128, 1], F32)
        nc.gpsimd.memset(eps_c[:], 1e-20)

        # qk with rows in partitions: qk_all[p, blk*128 + d] = qk[blk*128+p, d]
        qk_all = const_pool.tile([128, b_shard], F32)
        for blk in range(nblk):
            nc.scalar.dma_start(
                qk_all[:, blk * 128 : (blk + 1) * 128],
                t_qk.ap()[blk * 128 : (blk + 1) * 128, :],
            )

        def build_dq(bb):
            """Diag weights for block bb: dq[p, j, :] = qk[p, j] * I[p, :]."""
            qk_blk = qk_all[:, bb * 128 : bb * 128 + 128]
            dq_t = work_pool.tile([128, pe_d, 128], F32, tag="dq_t", bufs=dq_bufs)
            nc.vector.tensor_tensor(
                dq_t[:],
                qk_blk[:, :pe_d][:, :, None].to_broadcast([128, pe_d, 128]),
                ident[:][:, None, :].to_broadcast([128, pe_d, 128]),
                op=OP.mult,
            )
            return dq_t

        dq_cur = build_dq(0)

        def make_epilogue(bb, s_t, l2_t, split_out=False):
            """Emit softmax + top-k for block bb as two sub-bursts, dropped
            into the next block's chunk stream so the DVE queue never backs
            up far enough to block the streaming pipeline's buffer reuse."""
            state = {}

            def part1():
                # s is already scaled by 1/sqrt(d) (folded into qk on host)
                m_t = work_pool.tile([128, 1], F32, tag="m_t")
                nc.vector.tensor_reduce(m_t[:], s_t[:], axis=AX.X, op=OP.max)
                mb_t = work_pool.tile([128, 1], F32, tag="mb_t")
                nc.vector.tensor_scalar_mul(mb_t[:], m_t[:], -1.0)

                e_t = psum_pool.tile([128, N_CAND], F32, tag="e_t")
                sum_t = work_pool.tile([128, 1], F32, tag="sum_t")
                nc.scalar.activation(
                    e_t[:], s_t[:], AF.Exp, bias=mb_t[:], scale=1.0,
                    accum_out=sum_t[:],
                )
                r_t = work_pool.tile([128, 1], F32, tag="r_t")
                nc.vector.reciprocal(r_t[:], sum_t[:])
                r9_t = work_pool.tile([128, 1], F32, tag="r9_t")
                nc.vector.tensor_scalar_mul(r9_t[:], r_t[:], 1.0 - GAMMA)
                # p = e * (0.9/sum) + GAMMA/N_CAND  (in place in PSUM)
                nc.vector.tensor_scalar(
                    e_t[:], e_t[:], r9_t[:], MIX, op0=OP.mult, op1=OP.add
                )
                lp_t = psum_pool.tile([128, N_CAND], F32, tag="lp_t")
                nc.scalar.activation(lp_t[:], e_t[:], AF.Ln)

                # keys = log(p) + g = lp - l2  (write over s_t, now dead)
                nc.vector.tensor_sub(s_t[:], lp_t[:], l2_t[:])
                keys_t = s_t

                # top-32 round 0
                idx_t = work_pool.tile([128, K_OUT], U32, tag="idx_t")
                m8_t = work_pool.tile([128, 8], F32, tag="m8_t")
                nc.vector.max(out=m8_t[:], in_=keys_t[:])
                nc.vector.max_index(
                    out=idx_t[:, 0:8], in_max=m8_t[:], in_values=keys_t[:]
                )
                nc.vector.match_replace(
                    out=keys_t[:], in_to_replace=m8_t[:], in_values=keys_t[:],
                    imm_value=NEG_BIG,
                )
                if split_out:
                    nc.scalar.dma_start(
                        out_ap[bb * 128 : bb * 128 + 128, 0:8], idx_t[:, 0:8]
                    )
                state.update(keys_t=keys_t, idx_t=idx_t, m8_t=m8_t)

            def part2():
                keys_t, idx_t, m8_t = (
                    state["keys_t"], state["idx_t"], state["m8_t"]
                )
                for r in range(1, K_OUT // 8):
                    nc.vector.max(out=m8_t[:], in_=keys_t[:])
                    nc.vector.max_index(
                        out=idx_t[:, r * 8 : (r + 1) * 8],
                        in_max=m8_t[:],
                        in_values=keys_t[:],
                    )
                    if r < K_OUT // 8 - 1:
                        nc.vector.match_replace(
                            out=keys_t[:],
                            in_to_replace=m8_t[:],
                            in_values=keys_t[:],
                            imm_value=NEG_BIG,
                        )
                    if split_out:
                        nc.scalar.dma_start(
                            out_ap[bb * 128 : bb * 128 + 128, r * 8 : (r + 1) * 8],
                            idx_t[:, r * 8 : (r + 1) * 8],
                        )

                if not split_out:
                    nc.scalar.dma_start(
                        out_ap[bb * 128 : bb * 128 + 128, :], idx_t[:]
                    )

            return [part1, part2]

        pending_epi = []

        # ---------------- main loop over 128-row blocks ------------------------
        for bb in range(nblk):
            r0 = bb * 128
            u_t = work_pool.tile([128, N_CAND], F32, tag="u_t")
            nc.scalar.dma_start(u_t[:], u_ap[r0 : r0 + 128, :])

            # gumbel early (only needs u; ACT is idle during streaming):
            # l2 = log(-log(u + 1e-20) + 1e-20); keys later use g = -l2
            l1_t = psum_pool.tile([128, N_CAND], F32, tag="l1_t")
            nc.scalar.activation(l1_t[:], u_t[:], AF.Ln, bias=eps_c[:], scale=1.0)
            l2_t = u_t  # u is dead; keep l2 in SBUF (DVE reads one PSUM input max)
            nc.scalar.activation(l2_t[:], l1_t[:], AF.Ln, bias=eps_c[:], scale=-1.0)
            if bb == nblk - 1:
                # touch Exp now so the epilogue's exp finds its ACT table
                # loaded (the load otherwise lands in the post-DMA tail)
                warm_t = work_pool.tile([128, 1], F32, tag="warm_t")
                nc.scalar.activation(warm_t[:], eps_c[:], AF.Exp)

            s_t = work_pool.tile([128, N_CAND], F32, tag="s_t")
            qk_blk = qk_all[:, r0 : r0 + 128]
            dq_t = dq_cur

            # last block streams a finer tail so the final chunk is small and
            # PE-free (elementwise engines chase the DMA right to the end)
            if bb == nblk - 1 and nch == 128 and slab == 32:
                chunk_list = [(0, 128, True), (128, 128, True), (256, 128, True),
                              (384, 96, True), (480, 32, False)]
            else:
                chunk_list = [(ch * nch, nch, True) for ch in range(nchunks)]

            last_blk = bb == nblk - 1
            for ch, (n0, nch_c, use_pe) in enumerate(chunk_list):
                cand_t = cand_pool.tile([128, nch, D], F32, tag="cand_t")
                nc.sync.dma_start(
                    cand_t[:, :nch_c, :], cand_ap[r0 : r0 + 128, n0 : n0 + nch_c, :]
                )
                seg = s_t[:, n0 : n0 + nch_c]

                if use_pe:
                    # PE: partial scores over d < pe_d, accumulated in PSUM
                    ps_t = psum_pool.tile([128, nch], F32, tag="ps_mm", bufs=ps_bufs)
                    for dd in range(pe_d):
                        nc.tensor.matmul(
                            ps_t[:, :nch_c],
                            dq_t[:, dd, :],
                            cand_t[:, :nch_c, dd],
                            start=(dd == 0),
                            stop=(dd == pe_d - 1),
                        )

                    # GPSIMD/DVE multiply + DVE segmented reduce for d >= gd0
                    for hi in range(nch_c // slab):
                        h = hi * slab
                        tmp_t = work_pool.tile(
                            [128, slab, rem_d], F32, tag="tmp_t", bufs=tmp_bufs
                        )
                        mul_eng = nc.vector if hi < dve_slabs else nc.gpsimd
                        mul_eng.tensor_tensor(
                            tmp_t[:],
                            cand_t[:, h : h + slab, gd0:],
                            qk_blk[:, None, gd0:].to_broadcast([128, slab, rem_d]),
                            op=OP.mult,
                        )
                        nc.vector.tensor_reduce(
                            seg[:, h : h + slab], tmp_t[:], axis=AX.X, op=OP.add
                        )

                    # combine: seg += PE partial
                    nc.vector.tensor_tensor(seg, seg, ps_t[:, :nch_c], op=OP.add)
                else:
                    # PE-free tail chunk (32 cands): GPSIMD takes d < rem_d
                    # with a contiguous full-width tmp write, DVE the trailing
                    # pe_d-sized range; both reduced on DVE
                    d1 = rem_d
                    tmp_a = work_pool.tile(
                        [128, slab, rem_d], F32, tag="tmp_t", bufs=tmp_bufs
                    )
                    nc.gpsimd.tensor_tensor(
                        tmp_a[:],
                        cand_t[:, :nch_c, :d1],
                        qk_blk[:, None, :d1].to_broadcast([128, nch_c, d1]),
                        op=OP.mult,
                    )
                    tmp_b = work_pool.tile(
                        [128, slab, rem_d], F32, tag="tmp_t", bufs=tmp_bufs
                    )
                    nc.vector.tensor_tensor(
                        tmp_b[:, :, : D - d1],
                        cand_t[:, :nch_c, d1:],
                        qk_blk[:, None, d1:].to_broadcast([128, nch_c, D - d1]),
                        op=OP.mult,
                    )
                    nc.vector.tensor_reduce(
                        seg, tmp_a[:], axis=AX.X, op=OP.add
                    )
                    seg_b = work_pool.tile([128, slab], F32, tag="seg_b")
                    nc.vector.tensor_reduce(
                        seg_b[:, :nch_c], tmp_b[:, :, : D - d1], axis=AX.X,
                        op=OP.add,
                    )
                    nc.vector.tensor_tensor(seg, seg, seg_b[:, :nch_c], op=OP.add)

                # previous block's delayed epilogue parts drop in here, after
                # this block's streaming has refilled the DVE queue
                if ch >= 1 and pending_epi:
                    pending_epi.pop(0)()

            # prepare next block's weights right after this block's chunks so
            # the PE pipeline restarts immediately
            if bb + 1 < nblk:
                dq_cur = build_dq(bb + 1)

            while pending_epi:
                pending_epi.pop(0)()
            pending_epi = make_epilogue(bb, s_t, l2_t, split_out=(bb == nblk - 1))

        while pending_epi:
            pending_epi.pop(0)()

    nc.compile()
    return nc


_CACHE = {}


def _get_nc():
    if "nc" not in _CACHE:
        _CACHE["nc"] = build_nc()
    return _CACHE["nc"]


def make_in_maps(target_embed, candidate_embeds, Wq, bq, Wk, bk, u):
    target_embed = np.ascontiguousarray(np.asarray(target_embed, dtype=np.float32))
    candidate_embeds = np.ascontiguousarray(
        np.asarray(candidate_embeds, dtype=np.float32)
    )
    Wq = np.asarray(Wq, dtype=np.float32)
    bq = np.asarray(bq, dtype=np.float32)
    Wk = np.asarray(Wk, dtype=np.float32)
    u = np.ascontiguousarray(np.asarray(u, dtype=np.float32))

    # Host-side projection (tiny): qk = ((target @ Wq.T + bq) @ Wk) / sqrt(d)
    q = target_embed @ Wq.T + bq
    qk = np.ascontiguousarray(((q @ Wk) * INVSCALE).astype(np.float32))

    in_maps = []
    for c in range(N_CORES):
        lo, hi = c * B_SHARD, (c + 1) * B_SHARD
        in_maps.append(
            {
                "qk": qk[lo:hi],
                "candidate_embeds": candidate_embeds[lo:hi],
                "u": u[lo:hi],
            }
        )
    return in_maps


def kernel(
    target_embed, candidate_embeds, Wq, bq, Wk, bk, u
):  # full inputs -> full output
    nc = _get_nc()
    in_maps = make_in_maps(target_embed, candidate_embeds, Wq, bq, Wk, bk, u)
    res = run_bass_kernel_spmd(nc, in_maps, core_ids=list(range(N_CORES)))
    outs = [r["out"].astype(np.int32) for r in res.results]
    return np.concatenate(outs, axis=0)
